# revision 1
# baseline (speedup 1.0000x reference)
"""Trainium2 Bass kernel for nn_CrossAttention_Mirror.

Sharding: 8 cores = 4 batches x 2 H-halves (32 rows each).
Per core: conv1x1 -> cross-attn (transposed-logit softmax, fp32r matmuls)
-> conv3x3 (9 shifted matmuls) -> BN (AllReduce stats) + ReLU
-> pair AllGather -> self-attn -> conv3x3 -> BN + ReLU -> pred conv1x1.
Halo rows for the convs travel through small pair AllGathers with host-fed
selector masks so the compiled program is identical on every core.
Precision: fp32r (tf32-like) for q/k/logits and convs; bf16 for softmax
weights E and the value matrices (error diluted by gamma=0.1 residual).
Large per-attention operands (keys, transposed values) are staged in DRAM
and streamed back through small SBUF rings to fit the 224KB/partition SBUF.
"""

import os
import numpy as np
from contextlib import ExitStack

import concourse.bacc as bacc
import concourse.mybir as mybir
import concourse.tile as tile
from concourse.bass_utils import run_bass_kernel_spmd

F32 = mybir.dt.float32
F32R = mybir.dt.float32r
BF16 = mybir.dt.bfloat16
AF = mybir.ActivationFunctionType
ALU = mybir.AluOpType

B, CIN, C, H, W = 4, 512, 256, 64, 64
NLOC = 32 * W            # 2048 positions per core
NFULL = H * W            # 4096
CS = C + 2               # 258
NT = NLOC // 512         # 4 n-tiles
MBS = NFULL // 128       # 32 key blocks
MBG = MBS // 4           # 8 key groups of 512 positions
EPS = 1e-5
SHIFT1 = 45.0
SHIFT2 = 20.0
INV_CNT = 1.0 / (B * H * W)

ONES, SH1, SH2, SELA, SELB, SELC, SELD = range(7)

_built = {}


def _decl_inputs(nc):
    def d(name, shape):
        return nc.dram_tensor(name, list(shape), F32, kind="ExternalInput").ap()

    a = {}
    a["x_loc"] = d("x_loc", (CIN, NLOC))
    a["y_full"] = d("y_full", (CIN, NFULL))
    a["flow_q"] = d("flow_q", (2, NLOC))
    a["flow_f"] = d("flow_f", (2, NFULL))
    a["w1x1T"] = d("w1x1T", (CIN, C))
    a["b1x1c"] = d("b1x1c", (128, 2))
    for nm in ("caq", "cak", "cav"):
        a[nm + "T"] = d(nm + "T", (C, C))
    for nm in ("saq", "sak", "sav"):
        a[nm + "Tf"] = d(nm + "Tf", (C, CS))
        a[nm + "Tt"] = d(nm + "Tt", (2, CS))
    a["c1T"] = d("c1T", (9, C, C))
    a["c2a"] = d("c2a", (9, C, C))
    a["c2t"] = d("c2t", (9, 2, C))
    a["bn1g"] = d("bn1g", (128, 2))
    a["bn1b"] = d("bn1b", (128, 2))
    a["bn2g"] = d("bn2g", (128, 2))
    a["bn2b"] = d("bn2b", (128, 2))
    a["predT"] = d("predT", (C, 1))
    a["predb"] = d("predb", (1, 1))
    a["gam1"] = d("gam1", (1, 1))
    a["gam2"] = d("gam2", (1, 1))
    a["consts"] = d("consts", (128, 7))
    a["ones_row"] = d("ones_row", (1, 128))
    a["zeros34"] = d("zeros34", (128, 34))
    return a


def _emit(nc, tc, ctx, a, dbg):
    sync, act, dve, pe, gps = nc.sync, nc.scalar, nc.vector, nc.tensor, nc.gpsimd

    pw = ctx.enter_context(tc.tile_pool(name="pw", bufs=1))
    misc = ctx.enter_context(tc.tile_pool(name="misc", bufs=1))
    pdram = ctx.enter_context(tc.tile_pool(name="pdram", bufs=1, space="DRAM"))
    pE = ctx.enter_context(tc.tile_pool(name="pE", bufs=4))
    pscr = ctx.enter_context(tc.tile_pool(name="pscr", bufs=2))
    psL = ctx.enter_context(tc.tile_pool(name="psL", bufs=2, space="PSUM"))
    psO = ctx.enter_context(tc.tile_pool(name="psO", bufs=6, space="PSUM"))

    _ctr = [0]

    def lt(shape, dtype=F32):
        _ctr[0] += 1
        return psL.tile(shape, dtype, tag="l", name=f"pl{_ctr[0]}")

    def ot(shape, dtype=F32):
        _ctr[0] += 1
        return psO.tile(shape, dtype, tag="o", name=f"po{_ctr[0]}")

    def scr(shape, dtype=F32, tag="sm", bufs=None):
        _ctr[0] += 1
        return pscr.tile(shape, dtype, tag=tag, name=f"sc{_ctr[0]}", bufs=bufs)

    def ptile(pool, shape, dtype, tag, bufs):
        _ctr[0] += 1
        return pool.tile(shape, dtype, tag=tag, bufs=bufs, name=f"t{_ctr[0]}")

    # ---- persistent weights / consts ----
    w1 = pw.tile([128, 4, C], F32R)
    sync.dma_start(w1[:], a["w1x1T"].rearrange("(a p) m -> p a m", p=128).bitcast(F32R))
    b1 = pw.tile([128, 2], F32)
    sync.dma_start(b1[:], a["b1x1c"][:])
    caw = {}
    for nm in ("caq", "cak", "cav"):
        t = pw.tile([128, 2, C], F32R, name=nm)
        sync.dma_start(t[:], a[nm + "T"].rearrange("(a p) m -> p a m", p=128).bitcast(F32R))
        caw[nm] = t
    saw, sat = {}, {}
    for nm in ("saq", "sak", "sav"):
        t = pw.tile([128, 2, CS], F32R, name=nm + "f")
        sync.dma_start(t[:], a[nm + "Tf"].rearrange("(a p) m -> p a m", p=128).bitcast(F32R))
        saw[nm] = t
        td = scr([2, CS], F32, tag="sm")
        sync.dma_start(td[:], a[nm + "Tt"][:])
        tt = pw.tile([2, CS], BF16, name=nm + "t")
        dve.tensor_copy(tt[:], td[:])
        sat[nm] = tt
    bn1g = pw.tile([128, 2], F32); sync.dma_start(bn1g[:], a["bn1g"][:])
    bn1b = pw.tile([128, 2], F32); sync.dma_start(bn1b[:], a["bn1b"][:])
    bn2g = pw.tile([128, 2], F32); sync.dma_start(bn2g[:], a["bn2g"][:])
    bn2b = pw.tile([128, 2], F32); sync.dma_start(bn2b[:], a["bn2b"][:])
    predw = pw.tile([128, 2, 1], F32R)
    sync.dma_start(predw[:], a["predT"].rearrange("(a p) m -> p a m", p=128).bitcast(F32R))
    predb = pw.tile([1, 1], F32); sync.dma_start(predb[:], a["predb"][:])
    gam1 = pw.tile([1, 1], F32); sync.dma_start(gam1[:], a["gam1"][:])
    gam2 = pw.tile([1, 1], F32); sync.dma_start(gam2[:], a["gam2"][:])
    cst = pw.tile([128, 7], F32); sync.dma_start(cst[:], a["consts"][:])
    cstr = pw.tile([128, 1], F32R)
    sync.dma_start(cstr[:], a["consts"][:, 0:1].bitcast(F32R))
    onesr = pw.tile([1, 128], F32R)
    sync.dma_start(onesr[:], a["ones_row"].bitcast(F32R))
    z34 = pw.tile([128, 34], F32R)
    sync.dma_start(z34[:], a["zeros34"].bitcast(F32R))

    # ---- DRAM scratch for streamed operands ----
    k1d = pdram.tile([128, 2, NFULL], F32)
    k2d = pdram.tile([128, 2, NFULL], F32)
    vT2d = pdram.tile([128, MBS, CS], BF16)

    # ---- long-lived activations ----
    p1b = ctx.enter_context(tc.tile_pool(name="p1b", bufs=1))
    cross = p1b.tile([128, 2, NLOC], F32R)
    flowq = p1b.tile([2, NLOC], BF16)
    for i in range(NT):
        sm = scr([4, 512], F32, tag="sm")
        sync.dma_start(sm[0:2, :], a["flow_q"][:, i * 512:(i + 1) * 512])
        dve.tensor_copy(flowq[:, i * 512:(i + 1) * 512], sm[0:2, :])
    t3 = p1b.tile([128, 2, NLOC], F32R)
    st1 = misc.tile([128, 8], F32)
    sq1 = misc.tile([128, 8], F32)
    st2 = misc.tile([128, 8], F32)
    sq2 = misc.tile([128, 8], F32)

    p1a_cm = tc.tile_pool(name="p1a", bufs=1)
    p1a = p1a_cm.__enter__()
    xb = p1a.tile([128, 2, NLOC], F32R)
    vT1 = p1a.tile([128, MBS, C], BF16)
    t1p = [p1a.tile([128, 34, 66], F32R, name=f"t1p{i}") for i in range(2)]
    c1w = p1a.tile([128, 9, 2, C], F32R)
    sync.dma_start(c1w[:], a["c1T"].rearrange("t (a p) m -> p t a m", p=128).bitcast(F32R))

    def zero_pads(bufs, tailbuf=None):
        for tp in bufs:
            sync.dma_start(tp[:, :, 0], z34[:])
            sync.dma_start(tp[:, :, 65], z34[:])
        if tailbuf is not None:
            sync.dma_start(tailbuf[:, :, 0], z34[0:2, :])
            sync.dma_start(tailbuf[:, :, 65], z34[0:2, :])

    zero_pads(t1p)

    # ---- phase A: xb = w1x1 @ x + b ----
    xr = a["x_loc"].rearrange("(a p) n -> p a n", p=128)
    yr = a["y_full"].rearrange("(a p) n -> p a n", p=128)

    def load4(src, sl):
        ts = []
        for g in range(2):
            t = ptile(p1a, [128, 2, 512], F32R, "xy", 2)
            sync.dma_start(t[:], src[:, 2 * g:2 * g + 2, sl].bitcast(F32R))
            ts.append(t)
        return ts

    for nt in range(NT):
        sl = slice(nt * 512, (nt + 1) * 512)
        xts = load4(xr, sl)
        for cb in range(2):
            ps = lt([128, 512])
            for kb in range(4):
                pe.matmul(ps[:], w1[:, kb, cb * 128:(cb + 1) * 128],
                          xts[kb // 2][:, kb % 2, :], start=(kb == 0), stop=(kb == 3))
            act.activation(xb[:, cb, sl], ps[:], AF.Identity, bias=b1[:, cb:cb + 1])

    # ---- y path: yb chunk -> k1 chunk (to DRAM) + vT1 tiles ----
    for ch in range(8):
        sl = slice(ch * 512, (ch + 1) * 512)
        yts = load4(yr, sl)
        ybc = ptile(p1a, [128, 2, 512], F32R, "ybc", 3)
        for cb in range(2):
            ps = lt([128, 512])
            for kb in range(4):
                pe.matmul(ps[:], w1[:, kb, cb * 128:(cb + 1) * 128],
                          yts[kb // 2][:, kb % 2, :], start=(kb == 0), stop=(kb == 3))
            act.activation(ybc[:, cb, :], ps[:], AF.Identity, bias=b1[:, cb:cb + 1])
        kst = ptile(p1a, [128, 2, 512], F32R, "ybc", 3)
        for cb in range(2):
            ps = lt([128, 512])
            for kb in range(2):
                pe.matmul(ps[:], caw["cak"][:, kb, cb * 128:(cb + 1) * 128],
                          ybc[:, kb, :], start=(kb == 0), stop=(kb == 1))
            act.activation(kst[:, cb, :], ps[:], AF.Copy)
        sync.dma_start(k1d[:, :, sl], kst[:].bitcast(F32))
        for sub in range(4):
            mb = ch * 4 + sub
            csl = slice(sub * 128, (sub + 1) * 128)
            ps = lt([128, C])
            for kb in range(2):
                pe.matmul(ps[:], ybc[:, kb, csl], caw["cav"][:, kb, :],
                          start=(kb == 0), stop=(kb == 1))
            act.activation(vT1[:, mb, :], ps[:], AF.Copy)

    # ---- attention (shared emitter) ----
    def attention(nt, q_nt, qt_, kdram, kt_, vt_sel, two58, shift_col, gam, resid_fn):
        sl = slice(nt * 512, (nt + 1) * 512)
        eacc = scr([128, 512], F32R, tag="eacc")
        dt2 = ot([2, 512]) if two58 else None
        o0 = ot([128, 512])
        o1 = ot([128, 512])
        for mbg in range(MBG):
            gsl = slice(mbg * 512, (mbg + 1) * 512)
            kr = ptile(p1a if not two58 else p2a_holder[0], [128, 2, 512], F32R, "kr", 3)
            sync.dma_start(kr[:], kdram[:, :, gsl].bitcast(F32R))
            vr = vt_sel(mbg)
            for sub in range(4):
                mb = mbg * 4 + sub
                msl = slice(mb * 128, (mb + 1) * 128)
                ssl = slice(sub * 128, (sub + 1) * 128)
                lg = lt([128, 512])
                nkb = 3 if two58 else 2
                for kb in range(2):
                    pe.matmul(lg[:], kr[:, kb, ssl], q_nt[:, kb, :],
                              start=(kb == 0), stop=(kb == nkb - 1))
                if two58:
                    pe.matmul(lg[:], kt_[:, msl], qt_[:, sl], start=False, stop=True)
                _ctr[0] += 1
                E = pE.tile([128, 512], BF16, tag="E", name=f"E{_ctr[0]}")
                act.activation(E[:], lg[:], AF.Exp, bias=cst[:, shift_col:shift_col + 1])
                if mb == 0:
                    dve.tensor_copy(eacc[:], E[:])
                else:
                    dve.tensor_add(eacc[:], eacc[:].bitcast(F32), E[:])
                if two58:
                    pe.matmul(dt2[:], vr[:, sub, C:C + 2], E[:],
                              start=(mb == 0), stop=(mb == MBS - 1))
                pe.matmul(o0[:], vr[:, sub, 0:128], E[:],
                          start=(mb == 0), stop=(mb == MBS - 1))
                pe.matmul(o1[:], vr[:, sub, 128:256], E[:],
                          start=(mb == 0), stop=(mb == MBS - 1))
        dt = ot([1, 512])
        pe.matmul(dt[:], cstr[:], eacc[:], start=True, stop=True)
        smA = scr([1, 512], F32, tag="smA")
        dve.reciprocal(smA[:], dt[0:1, :])
        smB = scr([1, 512], F32R, tag="smB")
        dve.tensor_scalar_mul(smB[:], smA[:], gam[:])
        bcp = lt([128, 512])
        pe.matmul(bcp[:], onesr[:], smB[:], start=True, stop=True)
        bc = scr([128, 512], F32, tag="bc")
        act.activation(bc[:], bcp[:], AF.Copy)
        resid_fn(nt, o0, o1, dt2, bc)

    p2a_holder = [None]

    def resid1(nt, o0, o1, dt2, bc):
        s0 = nt * 8 + 1
        sl = slice(nt * 512, (nt + 1) * 512)
        for cb, op in ((0, o0), (1, o1)):
            tmp = scr([128, 512], F32, tag="tmp")
            dve.tensor_mul(tmp[:], op[:], bc[:])
            dve.tensor_add(t1p[cb][:, s0:s0 + 8, 1:65],
                           tmp[:].rearrange("p (r c) -> p r c", r=8),
                           xb[:, cb, sl].rearrange("p (r c) -> p r c", r=8))

    def vt1_sel(mbg):
        return vT1[:, mbg * 4:(mbg + 1) * 4, :]

    for nt in range(NT):
        sl = slice(nt * 512, (nt + 1) * 512)
        q_nt = ptile(p1a, [128, 2, 512], F32R, "q1r", 2)
        for cb in range(2):
            ps = lt([128, 512])
            for kb in range(2):
                pe.matmul(ps[:], caw["caq"][:, kb, cb * 128:(cb + 1) * 128],
                          xb[:, kb, sl], start=(kb == 0), stop=(kb == 1))
            act.activation(q_nt[:, cb, :], ps[:], AF.Copy)
        attention(nt, q_nt, None, k1d, None, vt1_sel, False, SH1, gam1, resid1)

    # ---- t1 halo exchange (pairs) ----
    def _edge_write(pad, e0, e1, selt):
        p = pad.shape[0]
        ta = scr([128, 64], F32, tag="ta")
        tb = scr([128, 64], F32, tag="tb")
        dve.tensor_scalar_mul(ta[0:p, :], e0[:, 1, :], selt[:, SELA:SELA + 1])
        dve.tensor_scalar_mul(tb[0:p, :], e1[:, 1, :], selt[:, SELB:SELB + 1])
        dve.tensor_add(pad[:, 0, 1:65], ta[0:p, :], tb[0:p, :])
        ta2 = scr([128, 64], F32, tag="ta")
        tb2 = scr([128, 64], F32, tag="tb")
        dve.tensor_scalar_mul(ta2[0:p, :], e0[:, 0, :], selt[:, SELC:SELC + 1])
        dve.tensor_scalar_mul(tb2[0:p, :], e1[:, 0, :], selt[:, SELD:SELD + 1])
        dve.tensor_add(pad[:, 33, 1:65], ta2[0:p, :], tb2[0:p, :])

    def edge_exchange(pads, tailpad, idx):
        nch = 2 * 128 + (2 if tailpad is not None else 0)
        bnc = pdram.tile([nch, 2, 64], F32, name=f"bnc{idx}")
        for cb in range(2):
            csl = slice(cb * 128, (cb + 1) * 128)
            sync.dma_start(bnc[csl, 0, :], pads[cb][:, 1, 1:65].bitcast(F32))
            sync.dma_start(bnc[csl, 1, :], pads[cb][:, 32, 1:65].bitcast(F32))
        if tailpad is not None:
            sync.dma_start(bnc[256:258, 0, :], tailpad[:, 1, 1:65].bitcast(F32))
            sync.dma_start(bnc[256:258, 1, :], tailpad[:, 32, 1:65].bitcast(F32))
        ag = pdram.tile([2, nch, 2, 64], F32, name=f"ag{idx}")
        gps.collective_compute("AllGather", ALU.bypass,
                               replica_groups=[[0, 1], [2, 3], [4, 5], [6, 7]],
                               ins=[bnc.opt()], outs=[ag.opt()])
        ed = scr([128, 2, 2, 2, 64], F32, tag="ed", bufs=1)
        for r in range(2):
            for w in range(2):
                sync.dma_start(ed[:, r, :, w, :],
                               ag[r, 0:256, w, :].rearrange("(a p) z -> p a z", p=128))
        for cb in range(2):
            _edge_write(pads[cb], ed[:, 0, cb, :, :], ed[:, 1, cb, :, :], cst)
        if tailpad is not None:
            edt = scr([2, 2, 2, 64], F32, tag="edt", bufs=1)
            sync.dma_start(edt[:], ag[:, 256:258, :, :].rearrange("r q w z -> q r w z"))
            _edge_write(tailpad, edt[:, 0, :, :], edt[:, 1, :, :], cst[0:2, :])

    edge_exchange(t1p, None, 1)

    if dbg:
        dd = nc.dram_tensor("d_xb", [128, 2, NLOC], F32, kind="ExternalOutput").ap()
        sync.dma_start(dd[:], xb[:].bitcast(F32))
        for i in range(2):
            dp = nc.dram_tensor(f"d_t1p{i}", [128, 34, 66], F32, kind="ExternalOutput").ap()
            sync.dma_start(dp[:], t1p[i][:].bitcast(F32))

    # ---- conv3x3 (9 shifted matmuls) + raw copy with stats ----
    def conv3x3(pads, tailpad, wfull, wtail, outtile, ssum, sqsum):
        for cb in range(2):
            for nt in range(NT):
                cp = lt([128, 8, 64])
                idx = 0
                ntap = 18 + (9 if tailpad is not None else 0)
                for tapy in range(3):
                    for tapx in range(3):
                        tap = tapy * 3 + tapx
                        r0 = nt * 8 + tapy
                        c0 = tapx
                        for kb in range(2):
                            pe.matmul(cp[:], wfull[:, tap, kb, cb * 128:(cb + 1) * 128],
                                      pads[kb][:, r0:r0 + 8, c0:c0 + 64],
                                      start=(idx == 0), stop=(idx == ntap - 1))
                            idx += 1
                        if tailpad is not None:
                            pe.matmul(cp[:], wtail[:, tap, cb * 128:(cb + 1) * 128],
                                      tailpad[:, r0:r0 + 8, c0:c0 + 64],
                                      start=False, stop=(idx == ntap - 1))
                            idx += 1
                sl = slice(nt * 512, (nt + 1) * 512)
                act.activation(outtile[:, cb, sl], cp[:].rearrange("p r c -> p (r c)"),
                               AF.Copy, accum_out=ssum[:, cb * 4 + nt:cb * 4 + nt + 1])
                sqs = scr([128, 512], F32, tag="sqs", bufs=1)
                act.activation(sqs[:], cp[:].rearrange("p r c -> p (r c)"),
                               AF.Square, accum_out=sqsum[:, cb * 4 + nt:cb * 4 + nt + 1])

    conv3x3(t1p, None, c1w, None, cross, st1, sq1)

    # ---- BN stats allreduce + normalize(relu) in place ----
    def bn_phase(outtile, ssum, sqsum, bng, bnb, idx):
        stat = misc.tile([128, 4], F32, name=f"stat{idx}")
        dve.reduce_sum(stat[:, 0:1], ssum[:, 0:4], axis=mybir.AxisListType.X)
        dve.reduce_sum(stat[:, 1:2], ssum[:, 4:8], axis=mybir.AxisListType.X)
        dve.reduce_sum(stat[:, 2:3], sqsum[:, 0:4], axis=mybir.AxisListType.X)
        dve.reduce_sum(stat[:, 3:4], sqsum[:, 4:8], axis=mybir.AxisListType.X)
        bb = pdram.tile([128, 4], F32, name=f"bnb{idx}")
        sync.dma_start(bb[:], stat[:])
        bo = pdram.tile([128, 4], F32, name=f"bno{idx}", addr_space="Shared")
        gps.collective_compute("AllReduce", ALU.add,
                               replica_groups=[[0, 1, 2, 3, 4, 5, 6, 7]],
                               ins=[bb.opt()], outs=[bo.opt()])
        gst = misc.tile([128, 4], F32, name=f"gst{idx}")
        sync.dma_start(gst[:], bo[:])
        mean = misc.tile([128, 2], F32, name=f"mean{idx}")
        dve.tensor_scalar_mul(mean[:], gst[:, 0:2], INV_CNT)
        var = misc.tile([128, 2], F32, name=f"var{idx}")
        dve.tensor_scalar_mul(var[:], gst[:, 2:4], INV_CNT)
        m2 = misc.tile([128, 2], F32, name=f"m2{idx}")
        dve.tensor_mul(m2[:], mean[:], mean[:])
        dve.tensor_sub(var[:], var[:], m2[:])
        dve.tensor_scalar_add(var[:], var[:], EPS)
        rv = misc.tile([128, 2], F32, name=f"rv{idx}")
        dve.reciprocal(rv[:], var[:])
        rstd = misc.tile([128, 2], F32, name=f"rstd{idx}")
        act.activation(rstd[:], rv[:], AF.Sqrt)
        scal = misc.tile([128, 2], F32, name=f"scal{idx}")
        dve.tensor_mul(scal[:], bng[:], rstd[:])
        shf = misc.tile([128, 2], F32, name=f"shf{idx}")
        dve.tensor_mul(shf[:], mean[:], scal[:])
        dve.tensor_sub(shf[:], bnb[:], shf[:])
        for cb in range(2):
            for nt in range(NT):
                sl = slice(nt * 512, (nt + 1) * 512)
                act.activation(outtile[:, cb, sl],
                               outtile[:, cb, sl].bitcast(F32), AF.Relu,
                               bias=shf[:, cb:cb + 1], scale=scal[:, cb:cb + 1])

    bn_phase(cross, st1, sq1, bn1g, bn1b, 1)
    
    # ---- AllGather cross_out (pairs) ----
    cbn = pdram.tile([128, 2, NLOC], F32)
    sync.dma_start(cbn[:], cross[:].bitcast(F32))
    ag2 = pdram.tile([2, 128, 2, NLOC], F32)
    gps.collective_compute("AllGather", ALU.bypass,
                           replica_groups=[[0, 1], [2, 3], [4, 5], [6, 7]],
                           ins=[cbn.opt()], outs=[ag2.opt()])

    # release phase-1 big pool, open phase-2 pool
    p1a_cm.__exit__(None, None, None)
    p2a_cm = tc.tile_pool(name="p2a", bufs=1)
    p2a = p2a_cm.__enter__()
    p2a_holder[0] = p2a
    c2w = p2a.tile([128, 9, 2, C], F32R)
    sync.dma_start(c2w[:], a["c2a"].rearrange("t (a p) m -> p t a m", p=128).bitcast(F32R))
    c2wt = p2a.tile([2, 9, C], F32R)
    sync.dma_start(c2wt[:], a["c2t"].rearrange("t q m -> q t m").bitcast(F32R))
    t2p = [p2a.tile([128, 34, 66], F32R, name=f"t2p{i}") for i in range(2)]
    t2pt = p2a.tile([2, 34, 66], F32R)
    zero_pads(t2p, t2pt)
    flowf = p2a.tile([2, NFULL], BF16)
    for i in range(8):
        sm = scr([4, 512], F32, tag="sm")
        sync.dma_start(sm[0:2, :], a["flow_f"][:, i * 512:(i + 1) * 512])
        dve.tensor_copy(flowf[:, i * 512:(i + 1) * 512], sm[0:2, :])
    q2t = p2a.tile([2, NLOC], BF16)
    k2t = p2a.tile([2, NFULL], BF16)

    # ---- q2 tail (bf16) from local cross/flow ----
    for nt in range(NT):
        sl = slice(nt * 512, (nt + 1) * 512)
        pst = lt([2, 512])
        for kb in range(2):
            pe.matmul(pst[:], saw["saq"][:, kb, C:CS], cross[:, kb, sl],
                      start=(kb == 0), stop=False)
        pe.matmul(pst[:], sat["saq"][:, C:CS], flowq[:, sl], start=False, stop=True)
        act.activation(q2t[:, sl], pst[:], AF.Copy)

    # ---- k2 / vT2 from gathered cat (k2, vT2 staged to DRAM) ----
    for ch in range(8):
        sl = slice(ch * 512, (ch + 1) * 512)
        half, loc = ch // 4, ch % 4
        ct = ptile(p2a, [128, 2, 512], F32R, "ct", 2)
        sync.dma_start(ct[:], ag2[half, :, :, loc * 512:(loc + 1) * 512].bitcast(F32R))
        kst = ptile(p2a, [128, 2, 512], F32R, "ct", 2)
        for cb in range(2):
            ps = lt([128, 512])
            for kb in range(2):
                pe.matmul(ps[:], saw["sak"][:, kb, cb * 128:(cb + 1) * 128],
                          ct[:, kb, :], start=(kb == 0), stop=False)
            pe.matmul(ps[:], sat["sak"][:, cb * 128:(cb + 1) * 128], flowf[:, sl],
                      start=False, stop=True)
            act.activation(kst[:, cb, :], ps[:], AF.Copy)
        sync.dma_start(k2d[:, :, sl], kst[:].bitcast(F32))
        pst = lt([2, 512])
        for kb in range(2):
            pe.matmul(pst[:], saw["sak"][:, kb, C:CS], ct[:, kb, :],
                      start=(kb == 0), stop=False)
        pe.matmul(pst[:], sat["sak"][:, C:CS], flowf[:, sl], start=False, stop=True)
        act.activation(k2t[:, sl], pst[:], AF.Copy)
        vst = ptile(p2a, [128, 4, CS], BF16, "vr", 3)
        for sub in range(4):
            mb = ch * 4 + sub
            csl = slice(sub * 128, (sub + 1) * 128)
            msl = slice(mb * 128, (mb + 1) * 128)
            ps = lt([128, CS])
            for kb in range(2):
                pe.matmul(ps[:], ct[:, kb, csl], saw["sav"][:, kb, :],
                          start=(kb == 0), stop=False)
            pe.matmul(ps[:], flowf[:, msl], sat["sav"][:], start=False, stop=True)
            act.activation(vst[:, sub, :], ps[:], AF.Copy)
        sync.dma_start(vT2d[:, ch * 4:(ch + 1) * 4, :], vst[:])

    # ---- attention 2 ----
    def resid2(nt, o0, o1, dt2, bc):
        s0 = nt * 8 + 1
        sl = slice(nt * 512, (nt + 1) * 512)
        for cb, op in ((0, o0), (1, o1)):
            tmp = scr([128, 512], F32, tag="tmp")
            dve.tensor_mul(tmp[:], op[:], bc[:])
            dve.tensor_add(t2p[cb][:, s0:s0 + 8, 1:65],
                           tmp[:].rearrange("p (r c) -> p r c", r=8),
                           cross[:, cb, sl].rearrange("p (r c) -> p r c", r=8))
        smC = scr([2, 512], F32, tag="smC")
        dve.tensor_mul(smC[:], dt2[0:2, :], bc[0:2, :])
        dve.tensor_add(t2pt[:, s0:s0 + 8, 1:65],
                       smC[:].rearrange("p (r c) -> p r c", r=8),
                       flowq[:, sl].rearrange("p (r c) -> p r c", r=8))

    def vt2_sel(mbg):
        vr = ptile(p2a, [128, 4, CS], BF16, "vr", 3)
        sync.dma_start(vr[:], vT2d[:, mbg * 4:(mbg + 1) * 4, :])
        return vr

    for nt in range(NT):
        sl = slice(nt * 512, (nt + 1) * 512)
        q_nt = ptile(p2a, [128, 2, 512], F32R, "q2r", 2)
        for cb in range(2):
            ps = lt([128, 512])
            for kb in range(2):
                pe.matmul(ps[:], saw["saq"][:, kb, cb * 128:(cb + 1) * 128],
                          cross[:, kb, sl], start=(kb == 0), stop=False)
            pe.matmul(ps[:], sat["saq"][:, cb * 128:(cb + 1) * 128], flowq[:, sl],
                      start=False, stop=True)
            act.activation(q_nt[:, cb, :], ps[:], AF.Copy)
        attention(nt, q_nt, q2t, k2d, k2t, vt2_sel, True, SH2, gam2, resid2)

    edge_exchange(t2p, t2pt, 2)

    if dbg:
        for i in range(2):
            dp = nc.dram_tensor(f"d_t2p{i}", [128, 34, 66], F32, kind="ExternalOutput").ap()
            sync.dma_start(dp[:], t2p[i][:].bitcast(F32))
        dp = nc.dram_tensor("d_t2pt", [2, 34, 66], F32, kind="ExternalOutput").ap()
        sync.dma_start(dp[:], t2pt[:].bitcast(F32))

    # ---- conv2 + BN2 + relu ----
    conv3x3(t2p, t2pt, c2w, c2wt, t3, st2, sq2)
    p2a_cm.__exit__(None, None, None)
    bn_phase(t3, st2, sq2, bn2g, bn2b, 2)
    
    # ---- pred 1x1 ----
    out_d = nc.dram_tensor("out_loc", [1, NLOC], F32, kind="ExternalOutput").ap()
    for nt in range(NT):
        sl = slice(nt * 512, (nt + 1) * 512)
        ps = lt([1, 512])
        for kb in range(2):
            pe.matmul(ps[:], predw[:, kb, :], t3[:, kb, sl],
                      start=(kb == 0), stop=(kb == 1))
        ob = scr([1, 512], F32, tag="ob")
        act.activation(ob[:], ps[:], AF.Identity, bias=predb[:])
        sync.dma_start(out_d[:, sl], ob[:])

    if dbg:
        for nm, t in (("d_cross", cross), ("d_t3", t3)):
            dd = nc.dram_tensor(nm, [128, 2, NLOC], F32, kind="ExternalOutput").ap()
            sync.dma_start(dd[:], t[:].bitcast(F32))


def _build(dbg=False):
    key = ("nc", dbg)
    if key in _built:
        return _built[key]
    nc = bacc.Bacc("TRN2", target_bir_lowering=False, debug=False, num_devices=8)
    a = _decl_inputs(nc)
    with tile.TileContext(nc) as tc, ExitStack() as ctx:
        _emit(nc, tc, ctx, a, dbg)
    nc.compile()
    _built[key] = nc
    return nc


def host_inputs(inputs):
    """Build the 8 per-core input dicts from the full problem inputs."""
    f = {k: np.asarray(v, dtype=np.float32) for k, v in inputs.items()}
    shared = {
        "w1x1T": np.ascontiguousarray(f["w1x1"].T),
        "b1x1c": np.ascontiguousarray(f["b1x1"].reshape(2, 128).T),
        "caqT": np.ascontiguousarray(f["ca_wq"].T),
        "cakT": np.ascontiguousarray(f["ca_wk"].T),
        "cavT": np.ascontiguousarray(f["ca_wv"].T),
        "c1T": np.ascontiguousarray(f["cbr1_w"].transpose(2, 3, 1, 0).reshape(9, C, C)),
        "c2a": np.ascontiguousarray(
            f["cbr2_w"].transpose(2, 3, 1, 0)[:, :, :C, :].reshape(9, C, C)),
        "c2t": np.ascontiguousarray(
            f["cbr2_w"].transpose(2, 3, 1, 0)[:, :, C:, :].reshape(9, 2, C)),
        "bn1g": np.ascontiguousarray(f["bn1_g"].reshape(2, 128).T),
        "bn1b": np.ascontiguousarray(f["bn1_b"].reshape(2, 128).T),
        "bn2g": np.ascontiguousarray(f["bn2_g"].reshape(2, 128).T),
        "bn2b": np.ascontiguousarray(f["bn2_b"].reshape(2, 128).T),
        "predT": np.ascontiguousarray(f["pred_w"].T),
        "predb": f["pred_b"].reshape(1, 1),
        "gam1": f["ca_gamma"].reshape(1, 1),
        "gam2": f["sa_gamma"].reshape(1, 1),
        "ones_row": np.ones((1, 128), np.float32),
        "zeros34": np.zeros((128, 34), np.float32),
    }
    for nm, w in (("saq", f["sa_wq"]), ("sak", f["sa_wk"]), ("sav", f["sa_wv"])):
        wT = np.ascontiguousarray(w.T)  # [in 258, out 258]
        shared[nm + "Tf"] = np.ascontiguousarray(wT[:C, :])
        shared[nm + "Tt"] = np.ascontiguousarray(wT[C:, :])
    in_maps = []
    for core in range(8):
        b, h = core // 2, core % 2
        sel = np.zeros((128, 7), np.float32)
        sel[:, ONES] = 1.0
        sel[:, SH1] = -SHIFT1
        sel[:, SH2] = -SHIFT2
        if h == 1:
            sel[:, SELA] = 1.0
        else:
            sel[:, SELD] = 1.0
        m = dict(shared)
        m["consts"] = sel
        m["x_loc"] = np.ascontiguousarray(
            f["x"][b, :, h * 32:(h + 1) * 32, :].reshape(CIN, NLOC))
        m["y_full"] = np.ascontiguousarray(f["y"][b].reshape(CIN, NFULL))
        m["flow_q"] = np.ascontiguousarray(
            f["flow"][b, :, h * 32:(h + 1) * 32, :].reshape(2, NLOC))
        m["flow_f"] = np.ascontiguousarray(f["flow"][b].reshape(2, NFULL))
        in_maps.append(m)
    return in_maps


def kernel(**inputs):
    dbg = bool(int(os.environ.get("BASS_KERNEL_DEBUG", "0")))
    nc = _build(dbg)
    in_maps = host_inputs(inputs)
    res = run_bass_kernel_spmd(nc, in_maps, core_ids=list(range(8)))
    out = np.empty((B, 1, H, W), np.float32)
    for core in range(8):
        b, h = core // 2, core % 2
        out[b, 0, h * 32:(h + 1) * 32, :] = res.results[core]["out_loc"].reshape(32, W)
    if dbg:
        kernel.debug_results = res.results
    return out



# revision 18
# speedup vs baseline: 2.0692x; 2.0692x over previous
"""Trainium2 Bass kernel for nn_CrossAttention_Mirror.

Sharding: 8 cores = 4 batches x 2 H-halves (32 rows each).
Per core: conv1x1 -> cross-attn (k1/vT1 SBUF-resident, fused y-path weights)
-> conv3x3 -> BN (stats AllGather + local sum) -> self-attn via Gram trick
(logits = cat_q^T (Wq^T Wk) cat_k, so raw normalized cross is the logits
stationary operand; no k2 projection) -> conv3x3 -> BN -> pred conv1x1.
The pair AllGather of raw (pre-BN) cross overlaps with the local-keys pass
of attention 2 (keys ordered mine-first via host-permuted flow tensors).
Edge halos travel through small pair AllGathers launched mid-attention.
Precision: bf16 activations/weights, f32r for attn1 q/k, fp32 PSUM/stats.
"""

import os
import numpy as np
from contextlib import ExitStack

import concourse.bacc as bacc
import concourse.mybir as mybir
import concourse.tile as tile
from concourse.bass_utils import run_bass_kernel_spmd

F32 = mybir.dt.float32
F32R = mybir.dt.float32r
BF16 = mybir.dt.bfloat16
FP8 = mybir.dt.float8e4
DR = mybir.MatmulPerfMode.DoubleRow
AF = mybir.ActivationFunctionType
ALU = mybir.AluOpType

B, CIN, C, H, W = 4, 512, 256, 64, 64
NLOC = 32 * W            # 2048 positions per core
NFULL = H * W            # 4096
CS = C + 2               # 258
NT = NLOC // 512         # 4 n-tiles
MBS = NFULL // 128       # 32 key blocks (16 local + 16 remote)
EPS = 1e-5
SHIFT1 = 45.0
SHIFT2 = 20.0
INV_CNT = 1.0 / (B * H * W)

ONES, SH1, SH2, SELA, SELB, SELC, SELD = range(7)

_built = {}


def _decl_inputs(nc):
    def d(name, shape, dt=F32):
        return nc.dram_tensor(name, list(shape), dt, kind="ExternalInput").ap()

    a = {}
    a["x_loc"] = d("x_loc", (CIN, NLOC), BF16)
    a["y_full"] = d("y_full", (CIN, NFULL), BF16)
    a["flow_f"] = d("flow_f", (2, NFULL), BF16)       # key order: mine|partner
    a["flow3"] = d("flow3", (3, NLOC), BF16)          # rows: zero, f0, f1 (local)
    a["w1x1T"] = d("w1x1T", (CIN, C), BF16)
    a["b1x1c"] = d("b1x1c", (128, 2))
    a["k1wT"] = d("k1wT", (CIN, C), BF16)
    a["k1bc"] = d("k1bc", (128, 2))
    a["v1wT"] = d("v1wT", (CIN, C), BF16)
    a["v1brow"] = d("v1brow", (1, C), BF16)
    a["caqT"] = d("caqT", (C, C))
    a["gqTf"] = d("gqTf", (C, CS), BF16)              # M[:256] rows
    a["gqTt"] = d("gqTt", (2, CS), BF16)              # M[256:] rows
    a["savTf"] = d("savTf", (C, CS + 1), BF16)        # out cols: v0..255, 0, v256, v257
    a["savTt"] = d("savTt", (2, CS + 1), BF16)
    a["vonerow"] = d("vonerow", (1, CS + 1), BF16)    # ones into col 256
    a["c1T"] = d("c1T", (9, C, C), BF16)
    a["c2a"] = d("c2a", (9, C, C), BF16)
    a["c2t"] = d("c2t", (9, 3, C), BF16)
    a["bn1g"] = d("bn1g", (128, 2))
    a["bn1b"] = d("bn1b", (128, 2))
    a["bn2g"] = d("bn2g", (128, 2))
    a["bn2b"] = d("bn2b", (128, 2))
    a["predT"] = d("predT", (C, 1), BF16)
    a["predb"] = d("predb", (1, 1))
    a["gam1"] = d("gam1", (1, 1))
    a["gam2"] = d("gam2", (1, 1))
    a["consts"] = d("consts", (128, 7))
    a["ones_row"] = d("ones_row", (1, 128))
    a["ones_bf"] = d("ones_bf", (1, 128), BF16)
    a["zeros34"] = d("zeros34", (128, 34), BF16)
    return a


def _emit(nc, tc, ctx, a, dbg):
    sync, act, dve, pe, gps = nc.sync, nc.scalar, nc.vector, nc.tensor, nc.gpsimd

    pw = ctx.enter_context(tc.tile_pool(name="pw", bufs=1))
    misc = ctx.enter_context(tc.tile_pool(name="misc", bufs=1))
    pdram = ctx.enter_context(tc.tile_pool(name="pdram", bufs=1, space="DRAM"))
    pE = ctx.enter_context(tc.tile_pool(name="pE", bufs=4))
    pscr = ctx.enter_context(tc.tile_pool(name="pscr", bufs=2))
    psL = ctx.enter_context(tc.tile_pool(name="psL", bufs=2, space="PSUM"))
    psO = ctx.enter_context(tc.tile_pool(name="psO", bufs=6, space="PSUM"))

    _ctr = [0]

    def lt(shape, dtype=F32):
        _ctr[0] += 1
        return psL.tile(shape, dtype, tag="l", name=f"pl{_ctr[0]}")

    def ot(shape, dtype=F32):
        _ctr[0] += 1
        return psO.tile(shape, dtype, tag="o", name=f"po{_ctr[0]}")

    def scr(shape, dtype=F32, tag="sm", bufs=None):
        _ctr[0] += 1
        return pscr.tile(shape, dtype, tag=tag, name=f"sc{_ctr[0]}", bufs=bufs)

    def ptile(pool, shape, dtype, tag, bufs):
        _ctr[0] += 1
        return pool.tile(shape, dtype, tag=tag, bufs=bufs, name=f"t{_ctr[0]}")

    def rearr4(ap):
        return ap.rearrange("(a p) m -> p a m", p=128)

    # ---- phase-A-critical weights first (DMA order ~ emission order) ----
    w1 = pw.tile([128, 4, C], BF16)
    sync.dma_start(w1[:], rearr4(a["w1x1T"]))
    b1 = pw.tile([128, 2], F32)
    sync.dma_start(b1[:], a["b1x1c"][:])

    p1b = ctx.enter_context(tc.tile_pool(name="p1b", bufs=1))
    p1aB_cm = tc.tile_pool(name="p1aB", bufs=1)
    p1aB = p1aB_cm.__enter__()
    p1aA_cm = tc.tile_pool(name="p1aA", bufs=1)
    p1aA = p1aA_cm.__enter__()
    xb = p1aA.tile([128, 2, NLOC], F32R)
    xr = a["x_loc"].rearrange("(a p) n -> p a n", p=128)
    yr = a["y_full"].rearrange("(a p) n -> p a n", p=128)

    # ---- phase A: xb = w1x1 @ x + b ----
    for nt in range(NT):
        sl = slice(nt * 512, (nt + 1) * 512)
        xt = ptile(p1aA, [128, 4, 512], BF16, "xt", 2)
        sync.dma_start(xt[:], xr[:, :, sl])
        for cb in range(2):
            ps = lt([128, 512])
            for kb in range(4):
                pe.matmul(ps[:], w1[:, kb, cb * 128:(cb + 1) * 128],
                          xt[:, kb, :], start=(kb == 0), stop=(kb == 3))
            act.activation(xb[:, cb, sl], ps[:], AF.Identity, bias=b1[:, cb:cb + 1])

    # ---- y path: k1 / vT1 directly from y (fused weights) ----
    k1w = pw.tile([128, 4, C], BF16)
    sync.dma_start(k1w[:], rearr4(a["k1wT"]))
    k1b = pw.tile([128, 2], F32)
    sync.dma_start(k1b[:], a["k1bc"][:])
    v1w = pw.tile([128, 4, C], BF16)
    sync.dma_start(v1w[:], rearr4(a["v1wT"]))
    v1brow = pw.tile([1, C], BF16)
    sync.dma_start(v1brow[:], a["v1brow"][:])
    onesbf = pw.tile([1, 128], BF16)
    sync.dma_start(onesbf[:], a["ones_bf"][:])
    caw = pw.tile([128, 2, C], F32R)
    sync.dma_start(caw[:], rearr4(a["caqT"]).bitcast(F32R))
    cst = pw.tile([128, 7], F32)
    sync.dma_start(cst[:], a["consts"][:])
    cstr = pw.tile([128, 1], F32R)
    sync.dma_start(cstr[:], a["consts"][:, 0:1].bitcast(F32R))
    onesr = pw.tile([1, 128], F32R)
    sync.dma_start(onesr[:], a["ones_row"].bitcast(F32R))
    z34 = pw.tile([128, 34], BF16)
    sync.dma_start(z34[:], a["zeros34"][:])

    k8 = p1aA.tile([128, 2, NFULL], FP8)
    ke8 = p1aA.tile([128, 2, NFULL], FP8)
    vT1 = p1aA.tile([128, MBS, C], BF16)
    for ch in range(8):
        sl = slice(ch * 512, (ch + 1) * 512)
        yt = ptile(p1aA, [128, 4, 512], BF16, "yt", 2)
        sync.dma_start(yt[:], yr[:, :, sl])
        for cb in range(2):
            ps = lt([128, 512])
            for kb in range(4):
                pe.matmul(ps[:], k1w[:, kb, cb * 128:(cb + 1) * 128],
                          yt[:, kb, :], start=(kb == 0), stop=(kb == 3))
            act.activation(k8[:, cb, sl], ps[:], AF.Identity, bias=k1b[:, cb:cb + 1])
            kf = scr([128, 512], BF16, tag="kf")
            act.activation(kf[:], ps[:], AF.Identity, bias=k1b[:, cb:cb + 1])
            dve.tensor_sub(ke8[:, cb, sl], kf[:], k8[:, cb, sl])
        for sub in range(4):
            mb = ch * 4 + sub
            csl = slice(sub * 128, (sub + 1) * 128)
            ps = lt([128, C])
            for kb in range(4):
                pe.matmul(ps[:], yt[:, kb, csl], v1w[:, kb, :],
                          start=(kb == 0), stop=False)
            pe.matmul(ps[:], onesbf[:], v1brow[:], start=False, stop=True)
            act.activation(vT1[:, mb, :], ps[:], AF.Copy)

    # prefetch conv1 weights + t1 pads while attention 1 runs
    c1w = pw.tile([128, 9, 2, C], BF16)
    sync.dma_start(c1w[:], a["c1T"].rearrange("t (a p) m -> p t a m", p=128))
    t1p = [p1aB.tile([128, 34, 66], BF16, name=f"t1p{i}") for i in range(2)]

    def zero_pads(bufs, tailbuf=None):
        for tp in bufs:
            sync.dma_start(tp[:, :, 0], z34[:])
            sync.dma_start(tp[:, :, 65], z34[:])
        if tailbuf is not None:
            sync.dma_start(tailbuf[:, :, 0], z34[0:3, :])
            sync.dma_start(tailbuf[:, :, 65], z34[0:3, :])

    zero_pads(t1p)

    # ---- halo exchange helper (selector masks, bf16 payload) ----
    def _edge_write(pad, e0, e1):
        p = pad.shape[0]
        ta = scr([128, 64], BF16, tag="ta")
        tb = scr([128, 64], BF16, tag="tb")
        dve.tensor_scalar_mul(ta[0:p, :], e0[:, 1, :], cst[0:p, SELA:SELA + 1])
        dve.tensor_scalar_mul(tb[0:p, :], e1[:, 1, :], cst[0:p, SELB:SELB + 1])
        dve.tensor_add(pad[:, 0, 1:65], ta[0:p, :], tb[0:p, :])
        ta2 = scr([128, 64], BF16, tag="ta")
        tb2 = scr([128, 64], BF16, tag="tb")
        dve.tensor_scalar_mul(ta2[0:p, :], e0[:, 0, :], cst[0:p, SELC:SELC + 1])
        dve.tensor_scalar_mul(tb2[0:p, :], e1[:, 0, :], cst[0:p, SELD:SELD + 1])
        dve.tensor_add(pad[:, 33, 1:65], ta2[0:p, :], tb2[0:p, :])

    def edge_exchange(pads, tailpad, idx):
        nch = 2 * 128 + (3 if tailpad is not None else 0)
        bnc = pdram.tile([nch, 2, 64], BF16, name=f"bnc{idx}")
        for cb in range(2):
            csl = slice(cb * 128, (cb + 1) * 128)
            sync.dma_start(bnc[csl, 0, :], pads[cb][:, 1, 1:65])
            sync.dma_start(bnc[csl, 1, :], pads[cb][:, 32, 1:65])
        if tailpad is not None:
            sync.dma_start(bnc[256:259, 0, :], tailpad[:, 1, 1:65])
            sync.dma_start(bnc[256:259, 1, :], tailpad[:, 32, 1:65])
        ag = pdram.tile([2, nch, 2, 64], BF16, name=f"ag{idx}")
        gps.collective_compute("AllGather", ALU.bypass,
                               replica_groups=[[0, 1], [2, 3], [4, 5], [6, 7]],
                               ins=[bnc.opt()], outs=[ag.opt()])
        ed = scr([128, 2, 2, 2, 64], BF16, tag="ed", bufs=1)
        for r in range(2):
            for w in range(2):
                sync.dma_start(ed[:, r, :, w, :],
                               ag[r, 0:256, w, :].rearrange("(a p) z -> p a z", p=128))
        for cb in range(2):
            _edge_write(pads[cb], ed[:, 0, cb, :, :], ed[:, 1, cb, :, :])
        if tailpad is not None:
            edt = scr([3, 2, 2, 64], BF16, tag="edt", bufs=1)
            sync.dma_start(edt[:], ag[:, 256:259, :, :].rearrange("r q w z -> q r w z"))
            _edge_write(tailpad, edt[:, 0, :, :], edt[:, 1, :, :])

    # ---- long-lived phase-boundary tiles ----
    cross = p1b.tile([128, 2, NLOC], BF16)   # raw conv1 out -> normalized local
    t3 = p1b.tile([128, 2, NLOC], BF16)
    flowf = p1b.tile([2, NFULL], BF16)
    sync.dma_start(flowf[:], a["flow_f"][:])
    flow3 = p1b.tile([3, NLOC], BF16)
    sync.dma_start(flow3[:], a["flow3"][:])
    st1 = misc.tile([128, 8], F32)
    sq1 = misc.tile([128, 8], F32)
    st2 = misc.tile([128, 8], F32)
    sq2 = misc.tile([128, 8], F32)

    # ---- conv3x3 (9 shifted matmuls) + raw copy with stats ----
    def conv3x3(pads, tailpad, wfull, wtail, outtile, ssum, sqsum, nts):
        for nt in nts:
            for cb in range(2):
                cp = lt([128, 8, 64])
                idx = 0
                ntap = 18 + (9 if tailpad is not None else 0)
                for tapy in range(3):
                    for tapx in range(3):
                        tap = tapy * 3 + tapx
                        r0 = nt * 8 + tapy
                        for kb in range(2):
                            pe.matmul(cp[:], wfull[:, tap, kb, cb * 128:(cb + 1) * 128],
                                      pads[kb][:, r0:r0 + 8, tapx:tapx + 64],
                                      start=(idx == 0), stop=(idx == ntap - 1))
                            idx += 1
                        if tailpad is not None:
                            pe.matmul(cp[:], wtail[:, tap, cb * 128:(cb + 1) * 128],
                                      tailpad[:, r0:r0 + 8, tapx:tapx + 64],
                                      start=False, stop=(idx == ntap - 1))
                            idx += 1
                sl = slice(nt * 512, (nt + 1) * 512)
                act.activation(outtile[:, cb, sl], cp[:].rearrange("p r c -> p (r c)"),
                               AF.Copy, accum_out=ssum[:, cb * 4 + nt:cb * 4 + nt + 1])
                sqs = scr([128, 512], F32, tag="sqs", bufs=1)
                act.activation(sqs[:], cp[:].rearrange("p r c -> p (r c)"),
                               AF.Square, accum_out=sqsum[:, cb * 4 + nt:cb * 4 + nt + 1])


    # ---- attention 1 (k1/vT1 resident; nt order 0,3,1,2) ----
    def bcast_inv(dcol, gam):
        # dcol [1,512] psum f32 -> bc [128,512] f32 = gam / d
        smA = scr([1, 512], F32, tag="smA")
        dve.reciprocal(smA[:], dcol[0:1, :])
        smB = scr([1, 512], F32R, tag="smB")
        dve.tensor_scalar_mul(smB[:], smA[:], gam[:])
        bcp = lt([128, 512])
        pe.matmul(bcp[:], onesr[:], smB[:], start=True, stop=True)
        bc = scr([128, 512], F32, tag="bc")
        act.activation(bc[:], bcp[:], AF.Copy)
        return bc

    gam1 = pw.tile([1, 1], F32); sync.dma_start(gam1[:], a["gam1"][:])
    gam2 = pw.tile([1, 1], F32); sync.dma_start(gam2[:], a["gam2"][:])

    for i, nt in enumerate((0, 3, 1, 2)):
        sl = slice(nt * 512, (nt + 1) * 512)
        q8 = ptile(p1aA, [128, 2, 512], FP8, "q1r", 2)
        qe8 = ptile(p1aA, [128, 2, 512], FP8, "q1e", 2)
        for cb in range(2):
            ps = lt([128, 512])
            for kb in range(2):
                pe.matmul(ps[:], caw[:, kb, cb * 128:(cb + 1) * 128],
                          xb[:, kb, sl], start=(kb == 0), stop=(kb == 1))
            act.activation(q8[:, cb, :], ps[:], AF.Copy)
            qf = scr([128, 512], BF16, tag="qf")
            act.activation(qf[:], ps[:], AF.Copy)
            dve.tensor_sub(qe8[:, cb, :], qf[:], q8[:, cb, :])
        eacc = scr([128, 512], F32R, tag="eacc")
        o0 = ot([128, 512])
        o1 = ot([128, 512])
        for mb in range(MBS):
            msl = slice(mb * 128, (mb + 1) * 128)
            lg = lt([128, 512])
            pe.matmul(lg[:], k8[:, :, msl], q8[:], perf_mode=DR,
                      start=True, stop=False)
            pe.matmul(lg[:], k8[:, :, msl], qe8[:], perf_mode=DR,
                      start=False, stop=False)
            pe.matmul(lg[:], ke8[:, :, msl], q8[:], perf_mode=DR,
                      start=False, stop=True)
            _ctr[0] += 1
            E = pE.tile([128, 512], BF16, tag="E", name=f"E{_ctr[0]}")
            act.activation(E[:], lg[:], AF.Exp, bias=cst[:, SH1:SH1 + 1])
            if mb == 0:
                dve.tensor_copy(eacc[:], E[:])
            else:
                dve.tensor_add(eacc[:], eacc[:].bitcast(F32), E[:])
            pe.matmul(o0[:], vT1[:, mb, 0:128], E[:],
                      start=(mb == 0), stop=(mb == MBS - 1))
            pe.matmul(o1[:], vT1[:, mb, 128:256], E[:],
                      start=(mb == 0), stop=(mb == MBS - 1))
        dt = ot([1, 512])
        pe.matmul(dt[:], cstr[:], eacc[:], start=True, stop=True)
        bc = bcast_inv(dt, gam1)
        s0 = nt * 8 + 1
        for cb, op in ((0, o0), (1, o1)):
            tmp = scr([128, 512], F32, tag="tmp")
            dve.tensor_mul(tmp[:], op[:], bc[:])
            dve.tensor_add(t1p[cb][:, s0:s0 + 8, 1:65],
                           tmp[:].rearrange("p (r c) -> p r c", r=8),
                           xb[:, cb, sl].rearrange("p (r c) -> p r c", r=8))
        if i == 1:
            edge_exchange(t1p, None, 1)

    if dbg:
        dd = nc.dram_tensor("d_xb", [128, 2, NLOC], F32, kind="ExternalOutput").ap()
        sync.dma_start(dd[:], xb[:].bitcast(F32))

    p1aA_cm.__exit__(None, None, None)

    conv3x3(t1p, None, c1w, None, cross, st1, sq1, (0, 1, 2, 3))
    p1aB_cm.__exit__(None, None, None)

    # ---- BN stats: local reduce -> AllGather -> local sum ----
    def bn_params(ssum, sqsum, bng, bnb, idx):
        stat = misc.tile([128, 4], F32, name=f"stat{idx}")
        dve.reduce_sum(stat[:, 0:1], ssum[:, 0:4], axis=mybir.AxisListType.X)
        dve.reduce_sum(stat[:, 1:2], ssum[:, 4:8], axis=mybir.AxisListType.X)
        dve.reduce_sum(stat[:, 2:3], sqsum[:, 0:4], axis=mybir.AxisListType.X)
        dve.reduce_sum(stat[:, 3:4], sqsum[:, 4:8], axis=mybir.AxisListType.X)
        bb = pdram.tile([128, 4], F32, name=f"bnb{idx}")
        sync.dma_start(bb[:], stat[:])
        bg = pdram.tile([8, 128, 4], F32, name=f"bng{idx}", addr_space="Shared")
        gps.collective_compute("AllGather", ALU.bypass,
                               replica_groups=[[0, 1, 2, 3, 4, 5, 6, 7]],
                               ins=[bb.opt()], outs=[bg.opt()])
        return bg

    def bn_finish(bg, bng, bnb, idx):
        gst = misc.tile([128, 8, 4], F32, name=f"gst{idx}")
        sync.dma_start(gst[:], bg.rearrange("w p s -> p w s"))
        tot = misc.tile([128, 4], F32, name=f"tot{idx}")
        for s in range(4):
            dve.reduce_sum(tot[:, s:s + 1], gst[:, :, s], axis=mybir.AxisListType.X)
        mean = misc.tile([128, 2], F32, name=f"mean{idx}")
        dve.tensor_scalar_mul(mean[:], tot[:, 0:2], INV_CNT)
        var = misc.tile([128, 2], F32, name=f"var{idx}")
        dve.tensor_scalar_mul(var[:], tot[:, 2:4], INV_CNT)
        m2 = misc.tile([128, 2], F32, name=f"m2{idx}")
        dve.tensor_mul(m2[:], mean[:], mean[:])
        dve.tensor_sub(var[:], var[:], m2[:])
        dve.tensor_scalar_add(var[:], var[:], EPS)
        rv = misc.tile([128, 2], F32, name=f"rv{idx}")
        dve.reciprocal(rv[:], var[:])
        rstd = misc.tile([128, 2], F32, name=f"rstd{idx}")
        act.activation(rstd[:], rv[:], AF.Sqrt)
        scal = misc.tile([128, 2], F32, name=f"scal{idx}")
        dve.tensor_mul(scal[:], bng[:], rstd[:])
        shf = misc.tile([128, 2], F32, name=f"shf{idx}")
        dve.tensor_mul(shf[:], mean[:], scal[:])
        dve.tensor_sub(shf[:], bnb[:], shf[:])
        return scal, shf

    bn1g = pw.tile([128, 2], F32); sync.dma_start(bn1g[:], a["bn1g"][:])
    bn1b = pw.tile([128, 2], F32); sync.dma_start(bn1b[:], a["bn1b"][:])
    bg1 = bn_params(st1, sq1, bn1g, bn1b, 1)

    # raw cross -> DRAM -> pair AllGather (overlaps with local-keys work).
    # The last 4 columns carry a dummy slice of the statsAG output so the
    # scheduler orders the small stats collective before this big one.
    cbn = pdram.tile([128, 2 * NLOC + 4], BF16)
    sync.dma_start(cbn[:, 0:2 * NLOC],
                   cross[:].rearrange("p a n -> p (a n)"))
    sync.dma_start(cbn[:, 2 * NLOC:], bg1.bitcast(BF16)[0, :, 0:4])
    agc = pdram.tile([2, 128, 2 * NLOC + 4], BF16)
    gps.collective_compute("AllGather", ALU.bypass,
                           replica_groups=[[0, 1], [2, 3], [4, 5], [6, 7]],
                           ins=[cbn.opt()], outs=[agc.opt()])

    scal1, shf1 = bn_finish(bg1, bn1g, bn1b, 1)

    def normalize(tile_, scal, shf, n):
        for j in range(n // 512):
            for cb in range(2):
                sl = slice(j * 512, (j + 1) * 512)
                act.activation(tile_[:, cb, sl], tile_[:, cb, sl], AF.Relu,
                               bias=shf[:, cb:cb + 1], scale=scal[:, cb:cb + 1])

    normalize(cross, scal1, shf1, NLOC)   # local half (after cbn DMA issued)

    # ---- phase 2 pool ----
    p2a_cm = tc.tile_pool(name="p2a", bufs=1)
    p2a = p2a_cm.__enter__()
    c2w = pw.tile([128, 9, 2, C], BF16)
    sync.dma_start(c2w[:], a["c2a"].rearrange("t (a p) m -> p t a m", p=128))
    c2wt = pw.tile([3, 9, C], BF16)
    sync.dma_start(c2wt[:], a["c2t"].rearrange("t q m -> q t m"))
    gqf = pw.tile([128, 2, CS], BF16)
    sync.dma_start(gqf[:], rearr4(a["gqTf"]))
    gqt = pw.tile([2, CS], BF16)
    sync.dma_start(gqt[:], a["gqTt"][:])
    savf = pw.tile([128, 2, CS + 1], BF16)
    sync.dma_start(savf[:], rearr4(a["savTf"]))
    savt = pw.tile([2, CS + 1], BF16)
    sync.dma_start(savt[:], a["savTt"][:])
    vone = pw.tile([1, CS + 1], BF16)
    sync.dma_start(vone[:], a["vonerow"][:])

    t2p = [p2a.tile([128, 34, 66], BF16, name=f"t2p{i}") for i in range(2)]
    t2pt = p2a.tile([3, 34, 66], BF16)
    zero_pads(t2p, t2pt)

    # q~ = M^T cat (local queries only)
    qt2 = p2a.tile([128, 2, NLOC], BF16)
    qt2t = p2a.tile([2, NLOC], BF16)
    for nt in range(NT):
        sl = slice(nt * 512, (nt + 1) * 512)
        for cb in range(2):
            csl = slice(cb * 128, (cb + 1) * 128)
            ps = lt([128, 512])
            for kb in range(2):
                pe.matmul(ps[:], gqf[:, kb, csl], cross[:, kb, sl],
                          start=(kb == 0), stop=False)
            pe.matmul(ps[:], gqt[:, csl], flowf[:, sl], start=False, stop=True)
            act.activation(qt2[:, cb, sl], ps[:], AF.Copy)
        pst = lt([2, 512])
        for kb in range(2):
            pe.matmul(pst[:], gqf[:, kb, C:CS], cross[:, kb, sl],
                      start=(kb == 0), stop=False)
        pe.matmul(pst[:], gqt[:, C:CS], flowf[:, sl], start=False, stop=True)
        act.activation(qt2t[:, sl], pst[:], AF.Copy)

    # vT2 for local key blocks (0..15)
    vT2 = p2a.tile([128, MBS, CS + 1], BF16)

    def vt2_block(mb, src, off):
        msl = slice((mb - off) * 128, (mb - off + 1) * 128)
        fsl = slice(mb * 128, (mb + 1) * 128)
        ps = lt([128, CS + 1])
        for kb in range(2):
            pe.matmul(ps[:], src[:, kb, msl], savf[:, kb, :],
                      start=(kb == 0), stop=False)
        pe.matmul(ps[:], flowf[:, fsl], savt[:], start=False, stop=False)
        pe.matmul(ps[:], onesbf[:], vone[:], start=False, stop=True)
        act.activation(vT2[:, mb, :], ps[:], AF.Copy)

    for mb in range(16):
        vt2_block(mb, cross, 0)

    # ---- attention 2: local-keys pass (gather in flight) ----
    oloc = p2a.tile([128, 2, NT, 512], F32)
    dloc = misc.tile([3, NT, 512], F32)

    def attn2_pass(nt, src, off, lo, hi, o0, o1, dt2):
        sl = slice(nt * 512, (nt + 1) * 512)
        for mb in range(lo, hi):
            msl = slice((mb - off) * 128, (mb - off + 1) * 128)
            fsl = slice(mb * 128, (mb + 1) * 128)
            lg = lt([128, 512])
            for kb in range(2):
                pe.matmul(lg[:], src[:, kb, msl], qt2[:, kb, sl],
                          start=(kb == 0), stop=False)
            pe.matmul(lg[:], flowf[:, fsl], qt2t[:, sl], start=False, stop=True)
            _ctr[0] += 1
            E = pE.tile([128, 512], BF16, tag="E", name=f"E{_ctr[0]}")
            act.activation(E[:], lg[:], AF.Exp, bias=cst[:, SH2:SH2 + 1])
            pe.matmul(o0[:], vT2[:, mb, 0:128], E[:],
                      start=(mb == lo), stop=(mb == hi - 1))
            pe.matmul(o1[:], vT2[:, mb, 128:256], E[:],
                      start=(mb == lo), stop=(mb == hi - 1))
            pe.matmul(dt2[:], vT2[:, mb, 256:259], E[:],
                      start=(mb == lo), stop=(mb == hi - 1))

    for nt in range(NT):
        o0, o1, dt2 = ot([128, 512]), ot([128, 512]), ot([3, 512])
        attn2_pass(nt, cross, 0, 0, 16, o0, o1, dt2)
        act.activation(oloc[:, 0, nt, :], o0[:], AF.Copy)
        act.activation(oloc[:, 1, nt, :], o1[:], AF.Copy)
        act.activation(dloc[:, nt, :], dt2[:], AF.Copy)

    # ---- remote half: mask-combine gathered raw, normalize, project ----
    rem = p2a.tile([128, 2, NLOC], BF16)
    ag1sb = p2a.tile([128, 2, NLOC], BF16)
    sync.dma_start(rem[:], agc[0, :, 0:2 * NLOC].rearrange("p (a n) -> p a n", a=2))
    sync.dma_start(ag1sb[:], agc[1, :, 0:2 * NLOC].rearrange("p (a n) -> p a n", a=2))
    dve.tensor_scalar_mul(rem[:], rem[:], cst[:, SELA:SELA + 1])
    dve.tensor_scalar_mul(ag1sb[:], ag1sb[:], cst[:, SELD:SELD + 1])
    dve.tensor_add(rem[:], rem[:], ag1sb[:])
    normalize(rem, scal1, shf1, NLOC)
    for mb in range(16, MBS):
        vt2_block(mb, rem, 16)

    # ---- attention 2: remote-keys pass + merge + resid ----
    for i, nt in enumerate((0, 3, 1, 2)):
        sl = slice(nt * 512, (nt + 1) * 512)
        o0, o1, dt2 = ot([128, 512]), ot([128, 512]), ot([3, 512])
        attn2_pass(nt, rem, 16, 16, MBS, o0, o1, dt2)
        dm = scr([3, 512], F32, tag="dm")
        dve.tensor_add(dm[:], dt2[:], dloc[:, nt, :])
        bc = bcast_inv(dm[0:1, :], gam2)
        s0 = nt * 8 + 1
        for cb, op in ((0, o0), (1, o1)):
            m = scr([128, 512], F32, tag="mm")
            dve.tensor_add(m[:], op[:], oloc[:, cb, nt, :])
            tmp = scr([128, 512], F32, tag="tmp")
            dve.tensor_mul(tmp[:], m[:], bc[:])
            dve.tensor_add(t2p[cb][:, s0:s0 + 8, 1:65],
                           tmp[:].rearrange("p (r c) -> p r c", r=8),
                           cross[:, cb, sl].rearrange("p (r c) -> p r c", r=8))
        smC = scr([3, 512], F32, tag="smC")
        dve.tensor_mul(smC[:], dm[:], bc[0:3, :])
        dve.tensor_add(t2pt[:, s0:s0 + 8, 1:65],
                       smC[:].rearrange("p (r c) -> p r c", r=8),
                       flow3[:, sl].rearrange("p (r c) -> p r c", r=8))
        if i == 1:
            edge_exchange(t2p, t2pt, 2)

    if dbg:
        ddl = nc.dram_tensor("d_dloc", [3, NT, 512], F32, kind="ExternalOutput").ap()
        sync.dma_start(ddl[:], dloc[:])
        ddq = nc.dram_tensor("d_qt2t", [2, NLOC], BF16, kind="ExternalOutput").ap()
        sync.dma_start(ddq[:], qt2t[:])
        def dump(nm, t, shape):
            dd = nc.dram_tensor(nm, shape, BF16, kind="ExternalOutput").ap()
            sync.dma_start(dd[:], t[:])
        for i in range(2):
            dump(f"d_t1p{i}", t1p[i], [128, 34, 66])
            dump(f"d_t2p{i}", t2p[i], [128, 34, 66])
        dump("d_t2pt", t2pt, [3, 34, 66])
        dump("d_cross", cross, [128, 2, NLOC])

    # ---- conv2 + BN2 + pred ----
    conv3x3(t2p, t2pt, c2w, c2wt, t3, st2, sq2, (0, 1, 2, 3))
    p2a_cm.__exit__(None, None, None)
    bn2g = pw.tile([128, 2], F32); sync.dma_start(bn2g[:], a["bn2g"][:])
    bn2b = pw.tile([128, 2], F32); sync.dma_start(bn2b[:], a["bn2b"][:])
    bg2 = bn_params(st2, sq2, bn2g, bn2b, 2)
    scal2, shf2 = bn_finish(bg2, bn2g, bn2b, 2)

    predw = pw.tile([128, 2, 1], BF16)
    sync.dma_start(predw[:], rearr4(a["predT"]))
    predb = pw.tile([1, 1], F32); sync.dma_start(predb[:], a["predb"][:])
    out_d = nc.dram_tensor("out_loc", [1, NLOC], F32, kind="ExternalOutput").ap()
    for nt in range(NT):
        sl = slice(nt * 512, (nt + 1) * 512)
        for cb in range(2):
            act.activation(t3[:, cb, sl], t3[:, cb, sl], AF.Relu,
                           bias=shf2[:, cb:cb + 1], scale=scal2[:, cb:cb + 1])
        ps = lt([1, 512])
        for kb in range(2):
            pe.matmul(ps[:], predw[:, kb, :], t3[:, kb, sl],
                      start=(kb == 0), stop=(kb == 1))
        ob = scr([1, 512], F32, tag="ob")
        act.activation(ob[:], ps[:], AF.Identity, bias=predb[:])
        sync.dma_start(out_d[:, sl], ob[:])

    if dbg:
        dd = nc.dram_tensor("d_t3", [128, 2, NLOC], BF16, kind="ExternalOutput").ap()
        sync.dma_start(dd[:], t3[:])


def _build(dbg=False):
    key = ("nc", dbg)
    if key in _built:
        return _built[key]
    nc = bacc.Bacc("TRN2", target_bir_lowering=False, debug=False, num_devices=8)
    a = _decl_inputs(nc)
    with tile.TileContext(nc) as tc, ExitStack() as ctx:
        _emit(nc, tc, ctx, a, dbg)
    nc.compile()
    _built[key] = nc
    return nc


def _sav_cols(w):
    # [in, 258] -> [in, 259] with out-col order v0..255, 0, v256, v257
    return np.concatenate(
        [w[:, :C], np.zeros((w.shape[0], 1), np.float32), w[:, C:]], axis=1)


def _vone():
    v = np.zeros((1, CS + 1), np.float32)
    v[0, C] = 1.0
    return v


def host_inputs(inputs):
    """Build the 8 per-core input dicts from the full problem inputs."""
    import ml_dtypes
    bf = ml_dtypes.bfloat16
    f = {k: np.asarray(v, dtype=np.float32) for k, v in inputs.items()}

    k1w = f["ca_wk"] @ f["w1x1"]          # [256, 512]
    v1w = f["ca_wv"] @ f["w1x1"]
    k1bias = f["ca_wk"] @ f["b1x1"]
    v1bias = f["ca_wv"] @ f["b1x1"]
    M = f["sa_wq"].T @ f["sa_wk"]         # [258, 258]

    shared = {
        "w1x1T": np.ascontiguousarray(f["w1x1"].T).astype(bf),
        "b1x1c": np.ascontiguousarray(f["b1x1"].reshape(2, 128).T),
        "k1wT": np.ascontiguousarray(k1w.T).astype(bf),
        "k1bc": np.ascontiguousarray(k1bias.reshape(2, 128).T),
        "v1wT": np.ascontiguousarray(v1w.T).astype(bf),
        "v1brow": v1bias.reshape(1, C).astype(bf),
        "caqT": np.ascontiguousarray(f["ca_wq"].T),
        "gqTf": np.ascontiguousarray(M[:C, :]).astype(bf),
        "gqTt": np.ascontiguousarray(M[C:, :]).astype(bf),
        "savTf": _sav_cols(f["sa_wv"].T[:C, :]).astype(bf),
        "savTt": _sav_cols(f["sa_wv"].T[C:, :]).astype(bf),
        "vonerow": _vone().astype(bf),
        "c1T": np.ascontiguousarray(
            f["cbr1_w"].transpose(2, 3, 1, 0).reshape(9, C, C)).astype(bf),
        "c2a": np.ascontiguousarray(
            f["cbr2_w"].transpose(2, 3, 1, 0)[:, :, :C, :].reshape(9, C, C)).astype(bf),
        "c2t": np.concatenate(
            [np.zeros((9, 1, C), np.float32),
             f["cbr2_w"].transpose(2, 3, 1, 0)[:, :, C:, :].reshape(9, 2, C)],
            axis=1).astype(bf),
        "bn1g": np.ascontiguousarray(f["bn1_g"].reshape(2, 128).T),
        "bn1b": np.ascontiguousarray(f["bn1_b"].reshape(2, 128).T),
        "bn2g": np.ascontiguousarray(f["bn2_g"].reshape(2, 128).T),
        "bn2b": np.ascontiguousarray(f["bn2_b"].reshape(2, 128).T),
        "predT": np.ascontiguousarray(f["pred_w"].T).astype(bf),
        "predb": f["pred_b"].reshape(1, 1),
        "gam1": f["ca_gamma"].reshape(1, 1),
        "gam2": f["sa_gamma"].reshape(1, 1),
        "ones_row": np.ones((1, 128), np.float32),
        "ones_bf": np.ones((1, 128), bf),
        "zeros34": np.zeros((128, 34), bf),
    }
    in_maps = []
    for core in range(8):
        b, h = core // 2, core % 2
        sel = np.zeros((128, 7), np.float32)
        sel[:, ONES] = 1.0
        sel[:, SH1] = -SHIFT1
        sel[:, SH2] = -SHIFT2
        if h == 1:
            sel[:, SELA] = 1.0
        else:
            sel[:, SELD] = 1.0
        m = dict(shared)
        m["consts"] = sel
        m["x_loc"] = np.ascontiguousarray(
            f["x"][b, :, h * 32:(h + 1) * 32, :].reshape(CIN, NLOC)).astype(bf)
        m["y_full"] = np.ascontiguousarray(f["y"][b].reshape(CIN, NFULL)).astype(bf)
        fmine = f["flow"][b, :, h * 32:(h + 1) * 32, :].reshape(2, NLOC)
        fpart = f["flow"][b, :, (1 - h) * 32:(2 - h) * 32, :].reshape(2, NLOC)
        ff = np.concatenate([fmine, fpart], axis=1)     # key order mine|partner
        m["flow_f"] = np.ascontiguousarray(ff).astype(bf)
        fl3 = np.zeros((3, NLOC), np.float32)
        fl3[1:3, :] = fmine
        m["flow3"] = fl3.astype(bf)
        in_maps.append(m)
    return in_maps


def assemble_output(results):
    out = np.empty((B, 1, H, W), np.float32)
    for core in range(8):
        b, h = core // 2, core % 2
        out[b, 0, h * 32:(h + 1) * 32, :] = results[core]["out_loc"].reshape(32, W)
    return out


def kernel(**inputs):
    dbg = bool(int(os.environ.get("BASS_KERNEL_DEBUG", "0")))
    nc = _build(dbg)
    in_maps = host_inputs(inputs)
    res = run_bass_kernel_spmd(nc, in_maps, core_ids=list(range(8)))
    if dbg:
        kernel.debug_results = res.results
    return assemble_output(res.results)
